# revision 18
# baseline (speedup 1.0000x reference)
"""CASCADES adapter (moe_routing) on 8 TRN2 NeuronCores.

Reference computation (B=4, S=2048, D=4096, R=8, K=4):
    centroid[b] = 0.7*x[b,-1] + 0.3*mean_s x[b,s]
    w[b]        = softmax(cos(centroid[b], keys) / 0.05)
    Lam[b]      = sum_k w[b,k] * pool[k]                 # [R,R]
    out[b,s]    = gate * (x[b,s] @ V^T) @ Lam[b]^T @ U^T

Sharding: core i handles batch i//2, sequence half i%2 (1024 rows).
The only cross-core dependency is the full-sequence centroid; each core
contributes 0.3/2048*seqsum_local (+0.7*x_last on odd cores, passed as a
host-prepared aux input) and a pairwise exchange of [128,32] (16 KB)
produces the centroid on both cores of each pair.

Everything parameter-only is folded on the host: gate into U, the K
mixing matrices Mk = gate*U @ pool[k] (stacked as Mall^T [32,4096]), and
key normalization. On device the output matmul contracts over 32
partitions: lhsT = w-scaled 4x-replicated x_V^T tile [32,128], rhs =
Mall^T chunk [32,512].

v2 changes vs baseline:
  - x / V^T / Mall / identity are fed as float32r directly (raw fp32
    bits; the PE truncates internally) -> transposes run at 1.5
    cycles/row instead of 2.0 and all on-device fp32->fp32r cast
    copies disappear.
  - sequence partial sums come from one fused tensor_reduce per s-tile
    on the drained SBUF tile ([128,32,128] -> [128,32]) instead of 64
    per-PSUM-slice reduces: ~17us vector instead of ~40us.
  - PSUM drains alternate scalar/vector in both phases.
  - collective selectable (AllReduce / AllGather+add) for A/B timing.
"""

import os
import numpy as np
from contextlib import ExitStack

B, S, D, R, K = 4, 2048, 4096, 8, 4
NCORES = 8
SH = S // 2            # rows per core
PT = 128               # partition tile
NT = SH // PT          # 8 sequence tiles per core
NCH = D // PT          # 32 d-chunks
KR = K * R             # 32

CC_KIND = os.environ.get("CASC_CC", "ar")   # ar | ag | none

_CACHE = {}
LAST_RESULTS = None


def _build_program():
    from concourse import bacc, tile, mybir, library_config

    dt = mybir.dt.float32
    f32r = mybir.dt.float32r
    add = mybir.AluOpType.add
    mult = mybir.AluOpType.mult
    AF = mybir.ActivationFunctionType
    AX = mybir.AxisListType

    nc = bacc.Bacc("TRN2", target_bir_lowering=False, debug=False,
                   num_devices=NCORES)

    xs = nc.dram_tensor("xs", [SH, D], f32r, kind="ExternalInput").ap()
    vt = nc.dram_tensor("vt", [PT, NCH * KR], f32r, kind="ExternalInput").ap()
    mall = nc.dram_tensor("mall", [KR, D], f32r, kind="ExternalInput").ap()
    kcols = nc.dram_tensor("kcols", [PT, K * NCH], dt, kind="ExternalInput").ap()
    aux = nc.dram_tensor("aux", [PT, NCH], dt, kind="ExternalInput").ap()
    ident = nc.dram_tensor("ident", [PT, PT], f32r, kind="ExternalInput").ap()
    mask = nc.dram_tensor("mask", [KR, K], dt, kind="ExternalInput").ap()
    if CC_KIND == "ag8":
        gmask = nc.dram_tensor(
            "gmask", [PT, NCORES * NCH], dt, kind="ExternalInput").ap()
    out = nc.dram_tensor("out", [SH, D], dt, kind="ExternalOutput").ap()

    NP_ = NT // 2      # s-tile pairs per core

    with tile.TileContext(nc) as tc, ExitStack() as c0:
        persist = c0.enter_context(tc.tile_pool(name="persist", bufs=1))
        dram = c0.enter_context(tc.tile_pool(name="dram", bufs=1, space="DRAM"))

        # ---- constants (gpsimd/SWDGE queue: don't block the x FIFO) ----
        kcols_sb = persist.tile([PT, K, NCH], dt, name="kcols_sb")
        nc.gpsimd.dma_start(kcols_sb[:], kcols[:])
        aux_sb = persist.tile([PT, NCH], dt, name="aux_sb")
        nc.gpsimd.dma_start(aux_sb[:], aux[:])
        ident_sb = persist.tile([PT, PT], f32r, name="ident_sb")
        nc.gpsimd.dma_start(ident_sb[:], ident[:])
        mask_sb = persist.tile([KR, K], dt, name="mask_sb")
        nc.gpsimd.dma_start(mask_sb[:], mask[:])
        ones_sb = persist.tile([PT, KR], dt, name="ones_sb")
        nc.vector.memset(ones_sb[:], 1.0)

        if CC_KIND == "rd":
            # pair exchange via direct SBUF->SBUF remote DMA (tpb XOR 1).
            # Allocation order is identical on every core (SPMD), so the
            # semaphore numbers agree across the pair.
            rsem = nc.alloc_semaphore("rd_rsem")
            lsem = nc.alloc_semaphore("rd_lsem")
            nc.gpsimd.sem_clear(rsem)
            nc.gpsimd.sem_clear(lsem)
            nc.gpsimd.load_library(library_config.remote_dma)
        vt_sb = persist.tile([PT, NCH, KR], f32r, name="vt_sb")
        nc.gpsimd.dma_start(vt_sb[:], vt[:].rearrange("p (c r) -> p c r", r=KR))
        mall_sb = persist.tile([KR, D], f32r, name="mall_sb")
        nc.gpsimd.dma_start(mall_sb[:], mall[:])

        # ---- persistent intermediates ----
        stash_sb = persist.tile([KR, NP_, 2 * PT], dt, name="stash_sb")
        seqparts = persist.tile([PT, NCH, NT], dt, name="seqparts")

        # ================= read phase =================
        with ExitStack() as c1:
            xin = c1.enter_context(tc.tile_pool(name="xin", bufs=3))
            xtp = c1.enter_context(
                tc.tile_pool(name="xtp", bufs=6, space="PSUM"))
            xts = c1.enter_context(tc.tile_pool(name="xts", bufs=3))
            xvp = c1.enter_context(
                tc.tile_pool(name="xvp", bufs=2, space="PSUM"))

            def emit_xv(pr, xt_all):
                # x_V^T (4x-replicated rows) for both tiles of the pair:
                # out[kr, sub*128+s], contraction over d in fp32r
                xv_ps = xvp.tile([KR, 2 * PT], dt, name="xv_ps")
                for ch in range(NCH):
                    nc.tensor.matmul(
                        xv_ps[:], vt_sb[:, ch, :], xt_all[:, ch, :],
                        start=(ch == 0), stop=(ch == NCH - 1))
                nc.scalar.copy(stash_sb[:, pr, :], xv_ps[:])

            pend = []   # (pr, xt_all) with x_V not yet emitted
            for pr in range(NP_):
                # xt_all[p, ch, sub*128+s] = x[pair rows]^T, f32r, d-major
                xt_all = xts.tile([PT, NCH, 2 * PT], f32r, name="xt_all")
                for sub in range(2):
                    t = 2 * pr + sub
                    xtile = xin.tile([PT, D], f32r, name="xtile")
                    # two half-tile loads so the first 16 chunks can be
                    # transposed while the rest streams in
                    nc.sync.dma_start(
                        xtile[:, 0:D // 2], xs[t * PT:(t + 1) * PT, 0:D // 2])
                    nc.sync.dma_start(
                        xtile[:, D // 2:D], xs[t * PT:(t + 1) * PT, D // 2:D])
                    for g in range(NCH // 4):
                        pt_ = xtp.tile([PT, 4, PT], f32r, name="pt_")
                        for j in range(4):
                            ch = 4 * g + j
                            nc.tensor.transpose(
                                pt_[:, j, :],
                                xtile[:, ch * PT:(ch + 1) * PT],
                                ident_sb[:],
                            )
                        # all drains on scalar so vector only does the
                        # per-PSUM-slice sequence reduces
                        nc.scalar.copy(
                            xt_all[:, 4 * g:4 * g + 4,
                                   sub * PT:(sub + 1) * PT],
                            pt_[:])
                        nc.vector.tensor_reduce(
                            seqparts[:, 4 * g:4 * g + 4, t], pt_[:],
                            axis=AX.X, op=add)

                # defer each pair's x_V matmuls so they never sit in the PE
                # queue ahead of a later tile's transposes (the routing
                # trigger only needs the transposes + reduces); emit just
                # late enough that the pool slot frees for pair pr+2
                pend.append((pr, xt_all))
                if pr >= 2:
                    emit_xv(*pend.pop(0))
            for args in pend:
                emit_xv(*args)

        # ================= routing =================
        cc_sb = persist.tile([PT, NCH], dt, name="cc_sb")
        nc.vector.tensor_reduce(cc_sb[:], seqparts[:], axis=AX.X, op=add)
        nc.vector.tensor_scalar_mul(cc_sb[:], cc_sb[:], 0.3 / S)
        nc.vector.tensor_add(cc_sb[:], cc_sb[:], aux_sb[:])

        c_sb = persist.tile([PT, NCH], dt, name="c_sb")
        if CC_KIND == "none":
            # timing-floor experiment: skip the exchange (wrong results)
            nc.vector.tensor_scalar_mul(c_sb[:], cc_sb[:], 2.0)
        elif CC_KIND == "ag8":
            # single-group 8-core AllGather (shared output): measured floor
            # ~5us vs ~22us for 2-core-group collectives. Each core then
            # masks out its own pair's two slots and sums them.
            cin = dram.tile([PT, NCH], dt, name="cin")
            nc.sync.dma_start(cin[:], cc_sb[:])
            cout = dram.tile([NCORES, PT, NCH], dt, name="cout",
                             addr_space="Shared")
            nc.gpsimd.collective_compute(
                "AllGather",
                mybir.AluOpType.bypass,
                replica_groups=[list(range(NCORES))],
                ins=[cin.opt()],
                outs=[cout.opt()],
            )
            gath = persist.tile([PT, NCORES, NCH], dt, name="gath")
            nc.sync.dma_start(gath[:], cout[:].rearrange("t p c -> p t c"))
            gm_sb = persist.tile([PT, NCORES, NCH], dt, name="gm_sb")
            nc.gpsimd.dma_start(
                gm_sb[:], gmask[:].rearrange("p (t c) -> p t c", c=NCH))
            gt = persist.tile([PT, NCORES, NCH], dt, name="gt")
            nc.vector.tensor_mul(gt[:], gath[:], gm_sb[:])
            g4 = persist.tile([PT, 4, NCH], dt, name="g4")
            nc.vector.tensor_add(g4[:], gt[:, 0:4, :], gt[:, 4:8, :])
            g2 = persist.tile([PT, 2, NCH], dt, name="g2")
            nc.vector.tensor_add(g2[:], g4[:, 0:2, :], g4[:, 2:4, :])
            nc.vector.tensor_add(c_sb[:], g2[:, 0, :], g2[:, 1, :])
        elif CC_KIND == "rd":
            peer_sb = persist.tile([PT, NCH], dt, name="peer_sb")
            nc.gpsimd.remote_dma_broadcast(
                peer_sb[:], cc_sb[:], rsem, lsem,
                rdests=[(0, 1)] + [None] * 7)
            nc.gpsimd.trigger_dma(None)
            nc.vector.wait_ge(rsem, 2)
            nc.vector.tensor_add(c_sb[:], cc_sb[:], peer_sb[:])
        else:
            cin = dram.tile([PT, NCH], dt, name="cin")
            nc.sync.dma_start(cin[:], cc_sb[:])
            if CC_KIND == "ar":
                cout = dram.tile([PT, NCH], dt, name="cout")
                nc.gpsimd.collective_compute(
                    "AllReduce",
                    add,
                    replica_groups=[[0, 1], [2, 3], [4, 5], [6, 7]],
                    ins=[cin.opt()],
                    outs=[cout.opt()],
                )
                nc.sync.dma_start(c_sb[:], cout[:])
            else:  # ag
                cout = dram.tile([2, PT, NCH], dt, name="cout")
                nc.gpsimd.collective_compute(
                    "AllGather",
                    mybir.AluOpType.bypass,
                    replica_groups=[[0, 1], [2, 3], [4, 5], [6, 7]],
                    ins=[cin.opt()],
                    outs=[cout.opt()],
                )
                peer_sb = persist.tile([PT, 2, NCH], dt, name="peer_sb")
                nc.sync.dma_start(
                    peer_sb[:], cout[:].rearrange("t p c -> p t c"))
                nc.vector.tensor_add(
                    c_sb[:], peer_sb[:, 0, :], peer_sb[:, 1, :])

        # per-partition partial dots: <c,kn_k> (k=0..3) and |c|^2
        partials = persist.tile([PT, K + 1], dt, name="partials")
        junk = persist.tile([PT, NCH], dt, name="junk")
        for k in range(K):
            nc.vector.tensor_mul(junk[:], c_sb[:], kcols_sb[:, k, :])
            nc.vector.tensor_reduce(
                partials[:, k:k + 1], junk[:], axis=AX.X, op=add)
        nc.vector.tensor_mul(junk[:], c_sb[:], c_sb[:])
        nc.vector.tensor_reduce(
            partials[:, K:K + 1], junk[:], axis=AX.X, op=add)

        with ExitStack() as cm, \
                tc.tile_pool(name="rps", bufs=1, space="PSUM") as rps:
            del cm
            r_ps = rps.tile([KR, K + 1], dt, name="r_ps")
            nc.tensor.matmul(r_ps[:], ones_sb[:], partials[:],
                             start=True, stop=True)
            rt_sb = persist.tile([KR, K + 1], dt, name="rt_sb")
            nc.scalar.copy(rt_sb[:], r_ps[:])

        cn = persist.tile([KR, 1], dt, name="cn")
        nc.scalar.sqrt(cn[:], rt_sb[:, K:K + 1])
        rcn = persist.tile([KR, 1], dt, name="rcn")
        nc.vector.reciprocal(rcn[:], cn[:])
        ex = persist.tile([KR, K], dt, name="ex")
        nc.vector.tensor_scalar(ex[:], rt_sb[:, 0:K], rcn[:], 1.0 / 0.05,
                                op0=mult, op1=mult)
        nc.scalar.activation(ex[:], ex[:], AF.Exp)
        ssum = persist.tile([KR, 1], dt, name="ssum")
        nc.vector.tensor_reduce(ssum[:], ex[:], axis=AX.X, op=add)
        rsum = persist.tile([KR, 1], dt, name="rsum")
        nc.vector.reciprocal(rsum[:], ssum[:])
        wmat = persist.tile([KR, K], dt, name="wmat")
        nc.vector.tensor_scalar_mul(wmat[:], ex[:], rsum[:])
        wcol = persist.tile([KR, 1], dt, name="wcol")
        junk2 = persist.tile([KR, K], dt, name="junk2")
        nc.vector.tensor_mul(junk2[:], wmat[:], mask_sb[:])
        nc.vector.tensor_reduce(wcol[:], junk2[:], axis=AX.X, op=add)

        # ================= write phase =================
        with ExitStack() as c2:
            otp = c2.enter_context(
                tc.tile_pool(name="otp", bufs=6, space="PSUM"))
            osb_pool = c2.enter_context(tc.tile_pool(name="osb", bufs=2))
            xvw_pool = c2.enter_context(tc.tile_pool(name="xvw", bufs=2))

            for t in range(NT):
                xvw = xvw_pool.tile([KR, PT], f32r, name="xvw")
                nc.scalar.mul(
                    xvw[:],
                    stash_sb[:, t // 2, (t % 2) * PT:(t % 2 + 1) * PT],
                    wcol[:])
                osb = osb_pool.tile([PT, D], dt, name="osb")
                for n in range(D // 512):
                    o_ps = otp.tile([PT, 512], dt, name="o_ps")
                    nc.tensor.matmul(
                        o_ps[:], xvw[:], mall_sb[:, n * 512:(n + 1) * 512],
                        start=True, stop=True)
                    dst = osb[:, n * 512:(n + 1) * 512]
                    if n % 2 == 0:
                        nc.scalar.copy(dst, o_ps[:])
                    else:
                        nc.vector.tensor_copy(dst, o_ps[:])
                # 4 finer stores per tile, split across two HWDGE queues so
                # the write stream starts as soon as the first chunks drain
                q = D // 4
                for i in range(4):
                    eng = nc.sync if i % 2 == 0 else nc.scalar
                    eng.dma_start(
                        out[t * PT:(t + 1) * PT, i * q:(i + 1) * q],
                        osb[:, i * q:(i + 1) * q])

    nc.compile()
    return nc


def _get_program():
    if "nc" not in _CACHE:
        _CACHE["nc"] = _build_program()
    return _CACHE["nc"]


def _host_prep(x, U, V, pool, keys, gate_w, gate_b):
    """Parameter-only folding + per-core shard/aux construction."""
    f32 = np.float32
    # gate (parameter-only)
    gin = np.concatenate([U.mean(axis=0), V.mean(axis=1)]).astype(f32)
    z = gin @ gate_w[0].astype(f32) + gate_b[0].astype(f32)
    gate = f32(1.0) / (f32(1.0) + np.exp(-z, dtype=f32))
    Ug = (gate * U).astype(f32)

    # Mall^T [32, 4096]: rows 8k+j = (gate*U @ pool[k])[:, j]
    mall = np.concatenate(
        [(Ug @ pool[k]).T.astype(f32) for k in range(K)], axis=0)
    mall = np.ascontiguousarray(mall, dtype=f32)

    # V^T in column-chunk layout, replicated 4x along r:
    # [p, c*KR + k*R + r] = V[r, c*128+p]
    vt = np.ascontiguousarray(
        np.tile(V.T.reshape(NCH, PT, R), (1, 1, K))
        .transpose(1, 0, 2).reshape(PT, NCH * KR),
        dtype=f32)

    # normalized keys in column layout [128, K*32]: [p, k*32+c] = kn[k, c*128+p]
    knorm = np.maximum(np.linalg.norm(keys, axis=1, keepdims=True), 1e-8)
    kn = (keys / knorm).astype(f32)
    kcols = np.ascontiguousarray(
        kn.reshape(K, NCH, PT).transpose(2, 0, 1).reshape(PT, K * NCH),
        dtype=f32)

    identity = np.eye(PT, dtype=f32)
    msk = np.zeros((KR, K), dtype=f32)
    for p in range(KR):
        msk[p, p // R] = 1.0

    shared = {"vt": vt, "mall": mall, "kcols": kcols, "ident": identity,
              "mask": msk}

    in_maps = []
    for core in range(NCORES):
        b, h = divmod(core, 2)
        xsrd = np.ascontiguousarray(x[b, h * SH:(h + 1) * SH, :], dtype=f32)
        if h == 1:
            aux = np.ascontiguousarray(
                (f32(0.7) * x[b, S - 1, :]).reshape(NCH, PT).T, dtype=f32)
        else:
            aux = np.zeros((PT, NCH), dtype=f32)
        m = {"xs": xsrd, "aux": aux, **shared}
        if CC_KIND == "ag8":
            gm = np.zeros((PT, NCORES, NCH), dtype=f32)
            gm[:, 2 * b, :] = 1.0
            gm[:, 2 * b + 1, :] = 1.0
            m["gmask"] = gm.reshape(PT, NCORES * NCH)
        in_maps.append(m)
    return in_maps


def kernel(x, U_shared, V_shared, core_pool, core_keys, gate_w, gate_b):
    global LAST_RESULTS
    from concourse import bass_utils

    x = np.asarray(x, dtype=np.float32)
    U = np.asarray(U_shared, dtype=np.float32)
    V = np.asarray(V_shared, dtype=np.float32)
    pool = np.asarray(core_pool, dtype=np.float32)
    keys = np.asarray(core_keys, dtype=np.float32)
    gw = np.asarray(gate_w, dtype=np.float32)
    gb = np.asarray(gate_b, dtype=np.float32)

    nc = _get_program()
    in_maps = _host_prep(x, U, V, pool, keys, gw, gb)
    res = bass_utils.run_bass_kernel_spmd(
        nc, in_maps, core_ids=list(range(NCORES)))
    LAST_RESULTS = res

    out = np.empty((B, S, D), dtype=np.float32)
    for core in range(NCORES):
        b, h = divmod(core, 2)
        out[b, h * SH:(h + 1) * SH, :] = res.results[core]["out"]
    return out
